# revision 12
# baseline (speedup 1.0000x reference)
"""MoE top-1 routed expert FFN (8 experts) on 8 Trainium2 NeuronCores.

Strategy: expert parallelism. Core e holds expert e's weights. The host
computes the token->expert permutation (top-1 dispatch is just a gather),
ships each core its tokens transposed (tokens on the matmul free dim),
and the device runs the whole FFN in transposed token space:

    hT = gelu_tanh(w1_tile.T @ xT + b1)        (per 128-wide ff tile)
    yT = sum_ff w2_tile.T @ hT + b2            (accumulated in PSUM)

so w1 ([D, FF]) and w2 ([FF, D]) act as PE stationary operands in their
natural layouts and no on-device transpose is needed. The host scatters
each core's yT back into the full output (tokens are disjoint across
experts, so the source's all-reduce degenerates to a scatter).

Schedule (from trace analysis): the PE must be continuously busy from the
end of the engine preamble or the HAM clock gate holds the PE at 1.2 GHz
(4096-cycle activity window; one idle gap resets it). A warmup matmul
burst bridges from preamble end to first-data-ready with no gap, sized so
the K=8/8 (2.4 GHz) flip happens inside the warmup. Input DMAs are queued
on the Sync HWDGE ring in exact PE consumption order with the first two
w1-halves split out so mm1(ff=0/1) unblock as early as possible; the bias
rides the otherwise-idle Scalar ring. The last chunk computes all mm1
tiles first (h tiles stay resident), then runs mm2 d-major so each PSUM
output bank finishes 24 ff-steps before the next and drains while the PE
keeps accumulating - the post-stream tail is one bias-add plus one DMA
instead of six.
"""

import os

import numpy as np

import concourse.mybir as mybir
import concourse.tile as tile
from concourse import bacc, bass_utils

N_CORES = 8
D = 768
FF = 3072
KD = D // 128  # 6
KF = FF // 128  # 24
NPACK = KF  # one ff-tile of (w1 slab | w2 tile) per DMA pack
# The 8-core PE clock is ~1.96 GHz steady / ~1.2 GHz during the HAM ramp
# window (~5.7us from first continuous PE activity). Real matmuls issued
# during the ramp make forward progress at ramp clock, so the warmup only
# needs to bridge preamble end -> first gate group ready; the early DMAs
# ride four rings in parallel so that bridge is as short as possible.
NWARM = int(os.environ.get("MOE_NWARM", "24"))
NFILL1 = int(os.environ.get("MOE_NFILL1", "2"))  # inside mm1(0,0), k2->k3
NFILL2 = int(os.environ.get("MOE_NFILL2", "2"))  # mm1(0,0) -> mm1(0,1)
NFILL3 = int(os.environ.get("MOE_NFILL3", "0"))  # mm1(0,1) -> mm2(0,0)
NSPLIT = 4  # packs 2..NSPLIT-1 ship as separate w1/w2 halves (JIT arrival)

_compiled = {}


def _maybe_trace():
    """Enable NTFF tracing only when MOE_TRACE=1 and the axon profile hook
    can be installed. The graded path never sets the env var."""
    if not os.environ.get("MOE_TRACE"):
        return False
    try:
        import sys
        import types

        if "antenv.axon_hooks" not in sys.modules:
            mod = types.ModuleType("antenv.axon_hooks")
            _h = [None]
            mod.set_axon_ntff_profile_hook = lambda h: _h.__setitem__(0, h)
            mod.get_axon_ntff_profile_hook = lambda: _h[0]
            sys.modules["antenv.axon_hooks"] = mod
            from trn_agent_boot.trn_boot import _ntff_profile_via_ctypes

            mod.set_axon_ntff_profile_hook(
                _ntff_profile_via_ctypes("/opt/axon/libaxon_pjrt.so")
            )
        return True
    except Exception:
        return False


def _build(chunks):
    """Build + compile the per-core FFN kernel for token chunk sizes `chunks`."""
    C = sum(chunks)
    f32 = mybir.dt.float32
    f16 = mybir.dt.float16
    gelu = mybir.ActivationFunctionType.Gelu_apprx_tanh
    ident = mybir.ActivationFunctionType.Identity

    nc = bacc.Bacc("TRN2", target_bir_lowering=False, debug=False, num_devices=N_CORES)
    # gate tensor: pack-0's w1 k-slabs fused with chunk-0's x, ordered as two
    # contiguous completion groups matching mm1(0,0)'s k-split: [w1h0 k0-2 |
    # x_c0 k0-2 | w1h0 k3-5 | x_c0 k3-5]. One fewer trigger and 2400B lines
    # lift the delivery-bound early DMA window vs three short-line transfers.
    c0 = chunks[0]
    gh = 384 + (KD // 2) * c0  # columns per gate completion group
    gp_d = nc.dram_tensor("gp", [128, 2 * gh + 1536], f16, kind="ExternalInput").ap()
    # xp[p, k*Cc + c] per chunk block, chunks 1.. only (chunk 0 rides gp)
    if len(chunks) > 1:
        xp_d = nc.dram_tensor(
            "xp", [128, KD * (C - c0)], f16, kind="ExternalInput"
        ).ap()
    # wp[ff]: [w1h(ff) | w2(ff)], each half a [128, 768] lhsT slab
    wp_d = nc.dram_tensor("wp", [NPACK, 128, 2 * D], f16, kind="ExternalInput").ap()
    # bp[:, :KF] = b1 tiles, bp[:, KF:KF+KD] = b2 tiles
    bp_d = nc.dram_tensor("bp", [128, KF + KD], f32, kind="ExternalInput").ap()
    yT_d = nc.dram_tensor("yT", [D, C], f16, kind="ExternalOutput").ap()

    last = len(chunks) - 1
    with tile.TileContext(nc) as tc:
        with (
            tc.tile_pool(name="wpool", bufs=1) as wpool,
            tc.tile_pool(name="xpool", bufs=1) as xpool,
            tc.tile_pool(name="hpool", bufs=4) as hpool,
            tc.tile_pool(name="h1pool", bufs=1) as h1pool,
            tc.tile_pool(name="ypool", bufs=6) as ypool,
            tc.tile_pool(name="bpool", bufs=1) as bpool,
            tc.tile_pool(name="phpool", bufs=2, space="PSUM") as phpool,
            tc.tile_pool(name="pypool", bufs=1, space="PSUM") as pypool,
        ):
            # PE warmup: dummy matmuls with no DMA dependency keep the PE busy
            # from preamble end until the first inputs land (an idle gap resets
            # the HAM activity window). Real matmuls take over as soon as data
            # is there: they make forward progress at ramp clock (~1.2 GHz), so
            # the warmup is sized to bridge only preamble end -> gate-1 ready.
            # The memset rides GpSimd, which is ready ~1us before DVE.
            warm_w = bpool.tile([128, 128], f16, tag="warm")
            nc.gpsimd.memset(warm_w[:], 0.0)
            warm_ps = phpool.tile([128, chunks[0]], f32, tag="ph", name="warm_ps")
            for _ in range(NWARM):
                nc.tensor.matmul(
                    warm_ps[:, :128], warm_w[:], warm_w[:], start=True, stop=True
                )

            # Early input DMAs ride FOUR rings in parallel (scalar/vector/
            # gpsimd/sync) so the first four consumption groups land nearly
            # simultaneously ~1.4us after the engine preambles, instead of
            # serially on one cold ring. The rest follow on Sync in exact PE
            # consumption order; x is packed chunk-major so only chunk 0's
            # slice gates the early stream, chunk 1's streams mid-flight.
            g_sb = xpool.tile([128, 2 * gh + 1536], f16, tag="g", name="g")
            if len(chunks) > 1:
                x_sb = xpool.tile([128, KD * (C - c0)], f16, tag="x", name="x")
            w_sb = [
                wpool.tile([128, 2 * D], f16, tag=f"wp{i}", name=f"wp{i}")
                for i in range(NPACK)
            ]
            b_sb = bpool.tile([128, KF + KD], f32, tag="b")
            # gate group 1 (w1h0 k0-2, x_c0 k0-2): scalar ring, first trigger,
            # nothing ahead of it in the queue. bias right behind (12KB).
            nc.scalar.dma_start(g_sb[:, :gh], gp_d[:, :gh])
            nc.scalar.dma_start(b_sb[:], bp_d)
            # preload the gelu ACT table now - after the triggers so it does
            # not delay g1, before the first real gelu (~1us after k5). The
            # Identity table loads lazily at the first drain, where ACT idles.
            warm_h = bpool.tile([128, 16], f16, tag="warmh")
            nc.scalar.activation(warm_h[:], warm_w[:, :16], gelu, bias=0.0, scale=1.0)
            # gate group 2 (k3-5), group 3 split (w1h1 | w2h0), and pack 1's
            # w2 half ride Sync in consumption order; packs 2/3 (split) ride
            # the slower GpSimd SWDGE ring whose need-times are later.
            nc.sync.dma_start(g_sb[:, gh : 2 * gh], gp_d[:, gh : 2 * gh])
            nc.sync.dma_start(g_sb[:, 2 * gh : 2 * gh + D], gp_d[:, 2 * gh : 2 * gh + D])
            nc.sync.dma_start(g_sb[:, 2 * gh + D :], gp_d[:, 2 * gh + D :])
            nc.sync.dma_start(w_sb[1][:, D:], wp_d[1, :, D:])
            nc.gpsimd.dma_start(w_sb[2][:, :D], wp_d[2, :, :D])
            nc.gpsimd.dma_start(w_sb[3][:, :D], wp_d[3, :, :D])
            nc.gpsimd.dma_start(w_sb[2][:, D:], wp_d[2, :, D:])
            nc.gpsimd.dma_start(w_sb[3][:, D:], wp_d[3, :, D:])
            for f in range(NSPLIT, NPACK):
                nc.sync.dma_start(w_sb[f][:], wp_d[f, :, :])
                if f == 13 and len(chunks) > 1:
                    nc.sync.dma_start(x_sb[:], xp_d)
            if NPACK <= 13 and len(chunks) > 1:
                nc.sync.dma_start(x_sb[:], xp_d)

            # chunks 0..last-1: software-pipelined mm1/mm2 stream (mm1 of step
            # i+1 issues before mm2 of step i so the gelu latency on ACT never
            # stalls the in-order PE queue). The last chunk contributes only
            # its mm1s here; its mm2 runs d-major afterwards.
            offs = [sum(chunks[:j]) for j in range(len(chunks))]
            steps = [(ci, ff) for ci in range(len(chunks)) for ff in range(KF)]
            py = {}  # PSUM output tiles, allocated lazily in first-use order
            h_tiles = {}

            def fill(n):
                for _ in range(n):
                    nc.tensor.matmul(
                        warm_ps[:, :128], warm_w[:], warm_w[:], start=True, stop=True
                    )

            def mm1(ci, ff):
                Cc = chunks[ci]
                wt = w_sb[ff]
                # last chunk's mm1-only phase runs at 0.68us/step, faster
                # than the ~0.85us matmul->gelu->slot-free chain, so a 2-deep
                # ph pool stalls every other step. Its py banks are idle then
                # (chunk-0's are drained, its own start later): rotate ph
                # through those 6 banks instead; first two stay in phpool so
                # no allocation waits on a chunk-0 drain still in flight.
                if ci == last and ff >= 2:
                    ph = pypool.tile(
                        [128, Cc], f32, tag=f"py{(ff - 2) % KD}", name=f"ph_{ci}_{ff}"
                    )
                else:
                    ph = phpool.tile([128, Cc], f32, tag="ph", name=f"ph_{ci}_{ff}")
                kh = KD // 2
                for k in range(KD):
                    # pack 0's w1 slabs and chunk 0's x live in the fused gate
                    # tile, split at gh into the two completion groups
                    if ff == 0:
                        go = (k - kh) * 128 + gh if k >= kh else k * 128
                        ws = g_sb[:, go : go + 128]
                    elif ff == 1:
                        go = 2 * gh + k * 128
                        ws = g_sb[:, go : go + 128]
                    else:
                        ws = wt[:, k * 128 : (k + 1) * 128]
                    if ci == 0:
                        go = 384 + ((k - kh) * Cc + gh if k >= kh else k * Cc)
                        xs = g_sb[:, go : go + Cc]
                    else:
                        xbase = KD * (offs[ci] - chunks[0])
                        xs = x_sb[:, xbase + k * Cc : xbase + (k + 1) * Cc]
                    nc.tensor.matmul(
                        ph[:], ws, xs, start=(k == 0), stop=(k == KD - 1)
                    )
                    if ci == 0 and ff == 0 and k == kh - 1:
                        # x half 2 still in flight: keep the PE busy so the
                        # HAM activity window never sees an idle gap
                        fill(NFILL1)
                if ci == last:
                    h_sb = h1pool.tile([128, Cc], f16, tag=f"h1_{ff}", name=f"h1_{ff}")
                else:
                    h_sb = hpool.tile([128, Cc], f16, tag="h", name=f"h_{ci}_{ff}")
                nc.scalar.activation(
                    h_sb[:], ph[:], gelu, bias=b_sb[:, ff : ff + 1], scale=1.0
                )
                h_tiles[(ci, ff)] = h_sb

            def mm2(ci, ff):
                wt = w_sb[ff]
                h_sb = h_tiles.pop((ci, ff))
                if ff == 0:
                    for d in range(KD):
                        py[(ci, d)] = pypool.tile(
                            [128, chunks[ci]], f32, tag=f"py{d}", name=f"py{d}_{ci}"
                        )
                for d in range(KD):
                    w2s = (
                        g_sb[:, 2 * gh + D + d * 128 : 2 * gh + D + (d + 1) * 128]
                        if ff == 0
                        else wt[:, D + d * 128 : D + (d + 1) * 128]
                    )
                    nc.tensor.matmul(
                        py[(ci, d)][:],
                        w2s,
                        h_sb[:],
                        start=(ff == 0),
                        stop=(ff == KF - 1),
                    )

            def y_drain_one(pyt, ci, d, use_act, q):
                Cc, c0 = chunks[ci], offs[ci]
                y_sb = ypool.tile([128, Cc], f16, tag="y", name=f"y_{ci}_{d}")
                b2ap = b_sb[:, KF + d : KF + d + 1]
                if use_act:
                    nc.scalar.activation(y_sb[:], pyt[:], ident, bias=b2ap)
                else:
                    nc.vector.tensor_scalar_add(y_sb[:], pyt[:], b2ap)
                q.dma_start(yT_d[d * 128 : (d + 1) * 128, c0 : c0 + Cc], y_sb[:])

            for idx, (ci, ff) in enumerate(steps):
                mm1(ci, ff)
                if idx == 0 and NFILL2:
                    fill(NFILL2)
                if idx == 1 and NFILL3:
                    fill(NFILL3)
                if idx > 0:
                    pci, pff = steps[idx - 1]
                    if pci != last:
                        mm2(pci, pff)
                        if pff == KF - 1:
                            # drain on DVE (idle) + alternate rings; ACT is
                            # busy with the next chunk's gelus
                            for d in range(KD):
                                y_drain_one(
                                    py[(pci, d)],
                                    pci,
                                    d,
                                    False,
                                    nc.sync if d % 2 == 0 else nc.scalar,
                                )

            # last chunk, d-major mm2: each py bank runs its full 24-step ff
            # accumulation back-to-back, then drains while the PE moves to the
            # next bank. Only the final bank's bias-add + DMA trail the stream.
            Cc = chunks[last]
            for d in range(KD):
                pyd = pypool.tile([128, Cc], f32, tag=f"py{d}", name=f"py{d}_last")
                for ff in range(KF):
                    w2s = (
                        g_sb[:, 2 * gh + D + d * 128 : 2 * gh + D + (d + 1) * 128]
                        if ff == 0
                        else w_sb[ff][:, D + d * 128 : D + (d + 1) * 128]
                    )
                    nc.tensor.matmul(
                        pyd[:],
                        w2s,
                        h_tiles[(last, ff)][:],
                        start=(ff == 0),
                        stop=(ff == KF - 1),
                    )
                # the very last drain is the tail critical path: split it in
                # half across ACT+DVE and two DMA rings so the post-stream
                # tail is ~half an add plus one (smaller) DMA
                if d == KD - 1:
                    half = (Cc // 2 + 7) // 8 * 8
                    c0t = offs[last]
                    y_sb = ypool.tile([128, Cc], f16, tag="y", name="y_tail")
                    b2ap = b_sb[:, KF + d : KF + d + 1]
                    nc.scalar.activation(y_sb[:, :half], pyd[:, :half], ident, bias=b2ap)
                    nc.vector.tensor_scalar_add(y_sb[:, half:], pyd[:, half:], b2ap)
                    nc.sync.dma_start(
                        yT_d[d * 128 : (d + 1) * 128, c0t : c0t + half], y_sb[:, :half]
                    )
                    nc.gpsimd.dma_start(
                        yT_d[d * 128 : (d + 1) * 128, c0t + half : c0t + Cc],
                        y_sb[:, half:],
                    )
                else:
                    y_drain_one(
                        pyd, last, d, d % 2 == 1, nc.sync if d % 2 == 0 else nc.scalar
                    )
    nc.compile()
    return nc


def _get_compiled(chunks):
    key = tuple(chunks)
    if key not in _compiled:
        _compiled[key] = _build(list(key))
    return _compiled[key]


def kernel(inputs, dispatch_order, w1, b1, w2, b2):
    x = np.asarray(inputs, dtype=np.float32)
    B, S, Dm = x.shape
    T = B * S
    xf = x.reshape(T, Dm)
    disp = np.asarray(dispatch_order).astype(np.int64)
    w1 = np.asarray(w1, dtype=np.float32)
    b1 = np.asarray(b1, dtype=np.float32)
    w2 = np.asarray(w2, dtype=np.float32)
    b2 = np.asarray(b2, dtype=np.float32)
    E = w1.shape[0]

    counts = np.bincount(disp, minlength=E)
    cmax = max(int(counts.max()), 16)
    # token capacity per core: near-equal chunks of <=512 (PSUM bank limit
    # for fp32 accumulation), multiples of 16, as small as cmax allows
    C = -(-cmax // 16) * 16
    n_chunks = -(-C // 512)
    chunks = []
    rem = C
    for j in range(n_chunks):
        c = -(-(rem // (n_chunks - j)) // 16) * 16
        chunks.append(c)
        rem -= c
    chunks.sort(reverse=True)

    order = np.argsort(disp, kind="stable")
    starts = np.concatenate([[0], np.cumsum(counts)])

    in_maps = []
    for e in range(E):
        ids = order[starts[e] : starts[e + 1]]
        xe = np.zeros((C, Dm), dtype=np.float32)
        xe[: len(ids)] = xf[ids]
        # chunk-major k-major packing per chunk
        xblk = [
            xe[o : o + cc].reshape(cc, KD, 128).transpose(2, 1, 0).reshape(128, KD * cc)
            for o, cc in zip([sum(chunks[:j]) for j in range(len(chunks))], chunks)
        ]
        # w1 in lhsT slab layout: w1h[ff][p, k*128+c] = w1[k*128+p, ff*128+c]
        w1h = (
            w1[e]
            .reshape(KD, 128, KF, 128)
            .transpose(2, 1, 0, 3)
            .reshape(KF, 128, KD * 128)
        )
        w2t = w2[e].reshape(KF, 128, D)
        wp = np.concatenate([w1h, w2t], axis=2)
        bp = np.concatenate(
            [b1[e].reshape(KF, 128).T, b2[e].reshape(KD, 128).T], axis=1
        )
        # gate tensor: [w1h0 k0-2 | x_c0 k0-2 | w1h0 k3-5 | x_c0 k3-5]
        h3 = (KD // 2) * chunks[0]
        gp = np.concatenate(
            [
                w1h[0][:, :384],
                xblk[0][:, :h3],
                w1h[0][:, 384:],
                xblk[0][:, h3:],
                w1h[1],
                w2t[0],
            ],
            axis=1,
        )
        im = {
            "gp": np.ascontiguousarray(gp).astype(np.float16),
            "wp": np.ascontiguousarray(wp).astype(np.float16),
            "bp": np.ascontiguousarray(bp),
        }
        if len(chunks) > 1:
            im["xp"] = np.ascontiguousarray(
                np.concatenate(xblk[1:], axis=1)
            ).astype(np.float16)
        in_maps.append(im)

    nc = _get_compiled(chunks)
    res = None
    for attempt in range(3):
        try:
            res = bass_utils.run_bass_kernel_spmd(
                nc, in_maps, core_ids=list(range(N_CORES)), trace=_maybe_trace()
            )
            break
        except Exception:
            # transient runtime/tunnel hiccups: retry a couple of times
            if attempt == 2:
                raise
            import time

            time.sleep(2.0)
    if res.exec_time_ns is not None:
        print(f"HW exec time: {res.exec_time_ns} ns")
        if res.instructions_and_trace is not None:
            print(f"trace: {res.instructions_and_trace[1]}")

    out = np.zeros((T, Dm), dtype=np.float32)
    for e in range(E):
        ids = order[starts[e] : starts[e + 1]]
        yT = res.results[e]["yT"]
        out[ids] = yT[:, : len(ids)].T.astype(np.float32)
    return out.reshape(B, S, Dm)



# revision 14
# speedup vs baseline: 1.0636x; 1.0636x over previous
"""MoE top-1 routed expert FFN (8 experts) on 8 Trainium2 NeuronCores.

Strategy: expert parallelism. Core e holds expert e's weights. The host
computes the token->expert permutation (top-1 dispatch is just a gather),
ships each core its tokens transposed (tokens on the matmul free dim),
and the device runs the whole FFN in transposed token space:

    hT = gelu_tanh(w1_tile.T @ xT + b1)        (per 128-wide ff tile)
    yT = sum_ff w2_tile.T @ hT + b2            (accumulated in PSUM)

so w1 ([D, FF]) and w2 ([FF, D]) act as PE stationary operands in their
natural layouts and no on-device transpose is needed. The host scatters
each core's yT back into the full output (tokens are disjoint across
experts, so the source's all-reduce degenerates to a scatter).

Schedule (from trace analysis): the PE must be continuously busy from the
end of the engine preamble or the HAM clock gate holds the PE at 1.2 GHz
(4096-cycle activity window; one idle gap resets it). A warmup matmul
burst bridges from preamble end to first-data-ready with no gap, sized so
the K=8/8 (2.4 GHz) flip happens inside the warmup. Input DMAs are queued
on the Sync HWDGE ring in exact PE consumption order with the first two
w1-halves split out so mm1(ff=0/1) unblock as early as possible; the bias
rides the otherwise-idle Scalar ring. The last chunk computes all mm1
tiles first (h tiles stay resident), then runs mm2 d-major so each PSUM
output bank finishes 24 ff-steps before the next and drains while the PE
keeps accumulating - the post-stream tail is one bias-add plus one DMA
instead of six.
"""

import os

import numpy as np

import concourse.mybir as mybir
import concourse.tile as tile
from concourse import bacc, bass_utils

N_CORES = 8
D = 768
FF = 3072
KD = D // 128  # 6
KF = FF // 128  # 24
NPACK = KF  # one ff-tile of (w1 slab | w2 tile) per DMA pack
# The 8-core PE clock is ~1.96 GHz steady / ~1.2 GHz during the HAM ramp
# window (~5.7us from first continuous PE activity). Real matmuls issued
# during the ramp make forward progress at ramp clock, so the warmup only
# needs to bridge preamble end -> first gate group ready; the early DMAs
# ride four rings in parallel so that bridge is as short as possible.
NWARM = int(os.environ.get("MOE_NWARM", "38"))
NFILL1 = int(os.environ.get("MOE_NFILL1", "8"))  # inside mm1(0,0), k2->k3
NFILL2 = int(os.environ.get("MOE_NFILL2", "0"))  # mm1(0,0) -> mm1(0,1)
NFILL3 = int(os.environ.get("MOE_NFILL3", "0"))  # mm1(0,1) -> mm2(0,0)
NSPLIT = 2  # packs 2..NSPLIT-1 ship as separate w1/w2 halves (JIT arrival)

_compiled = {}


def _maybe_trace():
    """Enable NTFF tracing only when MOE_TRACE=1 and the axon profile hook
    can be installed. The graded path never sets the env var."""
    if not os.environ.get("MOE_TRACE"):
        return False
    try:
        import sys
        import types

        if "antenv.axon_hooks" not in sys.modules:
            mod = types.ModuleType("antenv.axon_hooks")
            _h = [None]
            mod.set_axon_ntff_profile_hook = lambda h: _h.__setitem__(0, h)
            mod.get_axon_ntff_profile_hook = lambda: _h[0]
            sys.modules["antenv.axon_hooks"] = mod
            from trn_agent_boot.trn_boot import _ntff_profile_via_ctypes

            mod.set_axon_ntff_profile_hook(
                _ntff_profile_via_ctypes("/opt/axon/libaxon_pjrt.so")
            )
        return True
    except Exception:
        return False


def _build(chunks):
    """Build + compile the per-core FFN kernel for token chunk sizes `chunks`."""
    C = sum(chunks)
    f32 = mybir.dt.float32
    f16 = mybir.dt.float16
    gelu = mybir.ActivationFunctionType.Gelu_apprx_tanh
    ident = mybir.ActivationFunctionType.Identity

    nc = bacc.Bacc("TRN2", target_bir_lowering=False, debug=False, num_devices=N_CORES)
    # gate tensor: pack-0's w1 k-slabs fused with chunk-0's x, ordered as two
    # contiguous completion groups matching mm1(0,0)'s k-split: [w1h0 k0-2 |
    # x_c0 k0-2 | w1h0 k3-5 | x_c0 k3-5]. One fewer trigger and 2400B lines
    # lift the delivery-bound early DMA window vs three short-line transfers.
    c0 = chunks[0]
    gh = 384 + (KD // 2) * c0  # columns per gate completion group
    gp_d = nc.dram_tensor("gp", [128, 2 * gh + 1536], f16, kind="ExternalInput").ap()
    # xp[p, k*Cc + c] per chunk block, chunks 1.. only (chunk 0 rides gp)
    if len(chunks) > 1:
        xp_d = nc.dram_tensor(
            "xp", [128, KD * (C - c0)], f16, kind="ExternalInput"
        ).ap()
    # wp[ff]: [w1h(ff) | w2(ff)], each half a [128, 768] lhsT slab
    wp_d = nc.dram_tensor("wp", [NPACK, 128, 2 * D], f16, kind="ExternalInput").ap()
    # bp[:, :KF] = b1 tiles, bp[:, KF:KF+KD] = b2 tiles
    bp_d = nc.dram_tensor("bp", [128, KF + KD], f32, kind="ExternalInput").ap()
    yT_d = nc.dram_tensor("yT", [D, C], f16, kind="ExternalOutput").ap()

    last = len(chunks) - 1
    with tile.TileContext(nc) as tc:
        with (
            tc.tile_pool(name="wpool", bufs=1) as wpool,
            tc.tile_pool(name="xpool", bufs=1) as xpool,
            tc.tile_pool(name="hpool", bufs=4) as hpool,
            tc.tile_pool(name="h1pool", bufs=1) as h1pool,
            tc.tile_pool(name="ypool", bufs=6) as ypool,
            tc.tile_pool(name="bpool", bufs=1) as bpool,
            tc.tile_pool(name="phpool", bufs=2, space="PSUM") as phpool,
            tc.tile_pool(name="pypool", bufs=1, space="PSUM") as pypool,
        ):
            # PE warmup: dummy matmuls with no DMA dependency keep the PE busy
            # from preamble end until the first inputs land (an idle gap resets
            # the HAM activity window). Real matmuls take over as soon as data
            # is there: they make forward progress at ramp clock (~1.2 GHz), so
            # the warmup is sized to bridge only preamble end -> gate-1 ready.
            # The memset rides GpSimd, which is ready ~1us before DVE.
            warm_w = bpool.tile([128, 128], f16, tag="warm")
            nc.gpsimd.memset(warm_w[:], 0.0)
            warm_ps = phpool.tile([128, chunks[0]], f32, tag="ph", name="warm_ps")
            for _ in range(NWARM):
                nc.tensor.matmul(
                    warm_ps[:, :128], warm_w[:], warm_w[:], start=True, stop=True
                )

            # Early input DMAs ride FOUR rings in parallel (scalar/vector/
            # gpsimd/sync) so the first four consumption groups land nearly
            # simultaneously ~1.4us after the engine preambles, instead of
            # serially on one cold ring. The rest follow on Sync in exact PE
            # consumption order; x is packed chunk-major so only chunk 0's
            # slice gates the early stream, chunk 1's streams mid-flight.
            g_sb = xpool.tile([128, 2 * gh + 1536], f16, tag="g", name="g")
            if len(chunks) > 1:
                x_sb = xpool.tile([128, KD * (C - c0)], f16, tag="x", name="x")
            w_sb = [
                wpool.tile([128, 2 * D], f16, tag=f"wp{i}", name=f"wp{i}")
                for i in range(NPACK)
            ]
            b_sb = bpool.tile([128, KF + KD], f32, tag="b")
            # bias first on the scalar ring (12KB, needed by the first gelu),
            # then both ACT PWL tables preload behind it, well before use.
            nc.scalar.dma_start(b_sb[:], bp_d)
            warm_h = bpool.tile([128, 16], f16, tag="warmh")
            nc.scalar.activation(warm_h[:], warm_w[:, :16], gelu, bias=0.0, scale=1.0)
            nc.scalar.activation(warm_h[:], warm_w[:, :16], ident, bias=0.0, scale=1.0)
            # all input data on the Sync HWDGE ring in exact PE consumption
            # order (single warm queue = stable latency; parallel-ring variants
            # measured 2-3.5us trigger->data jitter that starves the stream):
            # the two gate groups (w1h0 k0-2 + x k0-2, then k3-5), the fused
            # third group [w1h1 | w2h0], then the packs with 2/1-split halves.
            nc.sync.dma_start(g_sb[:, :gh], gp_d[:, :gh])
            nc.sync.dma_start(g_sb[:, gh : 2 * gh], gp_d[:, gh : 2 * gh])
            nc.sync.dma_start(g_sb[:, 2 * gh :], gp_d[:, 2 * gh :])
            for f in range(2, NPACK):
                if f < NSPLIT:
                    nc.sync.dma_start(w_sb[f][:, :D], wp_d[f, :, :D])
                else:
                    nc.sync.dma_start(w_sb[f][:], wp_d[f, :, :])
                if 1 <= f - 1 < NSPLIT:
                    nc.sync.dma_start(w_sb[f - 1][:, D:], wp_d[f - 1, :, D:])
                if f == 13 and len(chunks) > 1:
                    nc.sync.dma_start(x_sb[:], xp_d)
            if NPACK <= 13 and len(chunks) > 1:
                nc.sync.dma_start(x_sb[:], xp_d)

            # chunks 0..last-1: software-pipelined mm1/mm2 stream (mm1 of step
            # i+1 issues before mm2 of step i so the gelu latency on ACT never
            # stalls the in-order PE queue). The last chunk contributes only
            # its mm1s here; its mm2 runs d-major afterwards.
            offs = [sum(chunks[:j]) for j in range(len(chunks))]
            steps = [(ci, ff) for ci in range(len(chunks)) for ff in range(KF)]
            py = {}  # PSUM output tiles, allocated lazily in first-use order
            h_tiles = {}

            def fill(n):
                for _ in range(n):
                    nc.tensor.matmul(
                        warm_ps[:, :128], warm_w[:], warm_w[:], start=True, stop=True
                    )

            def mm1(ci, ff):
                Cc = chunks[ci]
                wt = w_sb[ff]
                # last chunk's mm1-only phase runs at 0.68us/step, faster
                # than the ~0.85us matmul->gelu->slot-free chain, so a 2-deep
                # ph pool stalls every other step. Its py banks are idle then
                # (chunk-0's are drained, its own start later): rotate ph
                # through those 6 banks instead; first two stay in phpool so
                # no allocation waits on a chunk-0 drain still in flight.
                if ci == last and ff >= 2:
                    ph = pypool.tile(
                        [128, Cc], f32, tag=f"py{(ff - 2) % KD}", name=f"ph_{ci}_{ff}"
                    )
                else:
                    ph = phpool.tile([128, Cc], f32, tag="ph", name=f"ph_{ci}_{ff}")
                kh = KD // 2
                for k in range(KD):
                    # pack 0's w1 slabs and chunk 0's x live in the fused gate
                    # tile, split at gh into the two completion groups
                    if ff == 0:
                        go = (k - kh) * 128 + gh if k >= kh else k * 128
                        ws = g_sb[:, go : go + 128]
                    elif ff == 1:
                        go = 2 * gh + k * 128
                        ws = g_sb[:, go : go + 128]
                    else:
                        ws = wt[:, k * 128 : (k + 1) * 128]
                    if ci == 0:
                        go = 384 + ((k - kh) * Cc + gh if k >= kh else k * Cc)
                        xs = g_sb[:, go : go + Cc]
                    else:
                        xbase = KD * (offs[ci] - chunks[0])
                        xs = x_sb[:, xbase + k * Cc : xbase + (k + 1) * Cc]
                    nc.tensor.matmul(
                        ph[:], ws, xs, start=(k == 0), stop=(k == KD - 1)
                    )
                    if ci == 0 and ff == 0 and k == kh - 1:
                        # x half 2 still in flight: keep the PE busy so the
                        # HAM activity window never sees an idle gap
                        fill(NFILL1)
                if ci == last:
                    h_sb = h1pool.tile([128, Cc], f16, tag=f"h1_{ff}", name=f"h1_{ff}")
                else:
                    h_sb = hpool.tile([128, Cc], f16, tag="h", name=f"h_{ci}_{ff}")
                nc.scalar.activation(
                    h_sb[:], ph[:], gelu, bias=b_sb[:, ff : ff + 1], scale=1.0
                )
                h_tiles[(ci, ff)] = h_sb

            def mm2(ci, ff):
                wt = w_sb[ff]
                h_sb = h_tiles.pop((ci, ff))
                if ff == 0:
                    for d in range(KD):
                        py[(ci, d)] = pypool.tile(
                            [128, chunks[ci]], f32, tag=f"py{d}", name=f"py{d}_{ci}"
                        )
                for d in range(KD):
                    w2s = (
                        g_sb[:, 2 * gh + D + d * 128 : 2 * gh + D + (d + 1) * 128]
                        if ff == 0
                        else wt[:, D + d * 128 : D + (d + 1) * 128]
                    )
                    nc.tensor.matmul(
                        py[(ci, d)][:],
                        w2s,
                        h_sb[:],
                        start=(ff == 0),
                        stop=(ff == KF - 1),
                    )

            def y_drain_one(pyt, ci, d, use_act, q):
                Cc, c0 = chunks[ci], offs[ci]
                y_sb = ypool.tile([128, Cc], f16, tag="y", name=f"y_{ci}_{d}")
                b2ap = b_sb[:, KF + d : KF + d + 1]
                if use_act:
                    nc.scalar.activation(y_sb[:], pyt[:], ident, bias=b2ap)
                else:
                    nc.vector.tensor_scalar_add(y_sb[:], pyt[:], b2ap)
                q.dma_start(yT_d[d * 128 : (d + 1) * 128, c0 : c0 + Cc], y_sb[:])

            for idx, (ci, ff) in enumerate(steps):
                mm1(ci, ff)
                if idx == 0 and NFILL2:
                    fill(NFILL2)
                if idx == 1 and NFILL3:
                    fill(NFILL3)
                if idx > 0:
                    pci, pff = steps[idx - 1]
                    if pci != last:
                        mm2(pci, pff)
                        if pff == KF - 1:
                            # drain on DVE (idle) + alternate rings; ACT is
                            # busy with the next chunk's gelus
                            for d in range(KD):
                                y_drain_one(
                                    py[(pci, d)],
                                    pci,
                                    d,
                                    False,
                                    nc.sync if d % 2 == 0 else nc.scalar,
                                )

            # last chunk, d-major mm2: each py bank runs its full 24-step ff
            # accumulation back-to-back, then drains while the PE moves to the
            # next bank. Only the final bank's bias-add + DMA trail the stream.
            Cc = chunks[last]
            for d in range(KD):
                pyd = pypool.tile([128, Cc], f32, tag=f"py{d}", name=f"py{d}_last")
                for ff in range(KF):
                    w2s = (
                        g_sb[:, 2 * gh + D + d * 128 : 2 * gh + D + (d + 1) * 128]
                        if ff == 0
                        else w_sb[ff][:, D + d * 128 : D + (d + 1) * 128]
                    )
                    nc.tensor.matmul(
                        pyd[:],
                        w2s,
                        h_tiles[(last, ff)][:],
                        start=(ff == 0),
                        stop=(ff == KF - 1),
                    )
                # the very last drain is the tail critical path: split it in
                # half across ACT+DVE and two DMA rings so the post-stream
                # tail is ~half an add plus one (smaller) DMA
                if d == KD - 1:
                    half = (Cc // 2 + 7) // 8 * 8
                    c0t = offs[last]
                    y_sb = ypool.tile([128, Cc], f16, tag="y", name="y_tail")
                    b2ap = b_sb[:, KF + d : KF + d + 1]
                    nc.scalar.activation(y_sb[:, :half], pyd[:, :half], ident, bias=b2ap)
                    nc.vector.tensor_scalar_add(y_sb[:, half:], pyd[:, half:], b2ap)
                    nc.sync.dma_start(
                        yT_d[d * 128 : (d + 1) * 128, c0t : c0t + half], y_sb[:, :half]
                    )
                    nc.gpsimd.dma_start(
                        yT_d[d * 128 : (d + 1) * 128, c0t + half : c0t + Cc],
                        y_sb[:, half:],
                    )
                else:
                    y_drain_one(
                        pyd, last, d, d % 2 == 1, nc.sync if d % 2 == 0 else nc.scalar
                    )
    nc.compile()
    return nc


def _get_compiled(chunks):
    key = tuple(chunks)
    if key not in _compiled:
        _compiled[key] = _build(list(key))
    return _compiled[key]


def kernel(inputs, dispatch_order, w1, b1, w2, b2):
    x = np.asarray(inputs, dtype=np.float32)
    B, S, Dm = x.shape
    T = B * S
    xf = x.reshape(T, Dm)
    disp = np.asarray(dispatch_order).astype(np.int64)
    w1 = np.asarray(w1, dtype=np.float32)
    b1 = np.asarray(b1, dtype=np.float32)
    w2 = np.asarray(w2, dtype=np.float32)
    b2 = np.asarray(b2, dtype=np.float32)
    E = w1.shape[0]

    counts = np.bincount(disp, minlength=E)
    cmax = max(int(counts.max()), 16)
    # token capacity per core: near-equal chunks of <=512 (PSUM bank limit
    # for fp32 accumulation), multiples of 16, as small as cmax allows
    C = -(-cmax // 16) * 16
    n_chunks = -(-C // 512)
    chunks = []
    rem = C
    for j in range(n_chunks):
        c = -(-(rem // (n_chunks - j)) // 16) * 16
        chunks.append(c)
        rem -= c
    chunks.sort(reverse=True)

    order = np.argsort(disp, kind="stable")
    starts = np.concatenate([[0], np.cumsum(counts)])

    in_maps = []
    for e in range(E):
        ids = order[starts[e] : starts[e + 1]]
        xe = np.zeros((C, Dm), dtype=np.float32)
        xe[: len(ids)] = xf[ids]
        # chunk-major k-major packing per chunk
        xblk = [
            xe[o : o + cc].reshape(cc, KD, 128).transpose(2, 1, 0).reshape(128, KD * cc)
            for o, cc in zip([sum(chunks[:j]) for j in range(len(chunks))], chunks)
        ]
        # w1 in lhsT slab layout: w1h[ff][p, k*128+c] = w1[k*128+p, ff*128+c]
        w1h = (
            w1[e]
            .reshape(KD, 128, KF, 128)
            .transpose(2, 1, 0, 3)
            .reshape(KF, 128, KD * 128)
        )
        w2t = w2[e].reshape(KF, 128, D)
        wp = np.concatenate([w1h, w2t], axis=2)
        bp = np.concatenate(
            [b1[e].reshape(KF, 128).T, b2[e].reshape(KD, 128).T], axis=1
        )
        # gate tensor: [w1h0 k0-2 | x_c0 k0-2 | w1h0 k3-5 | x_c0 k3-5]
        h3 = (KD // 2) * chunks[0]
        gp = np.concatenate(
            [
                w1h[0][:, :384],
                xblk[0][:, :h3],
                w1h[0][:, 384:],
                xblk[0][:, h3:],
                w1h[1],
                w2t[0],
            ],
            axis=1,
        )
        im = {
            "gp": np.ascontiguousarray(gp).astype(np.float16),
            "wp": np.ascontiguousarray(wp).astype(np.float16),
            "bp": np.ascontiguousarray(bp),
        }
        if len(chunks) > 1:
            im["xp"] = np.ascontiguousarray(
                np.concatenate(xblk[1:], axis=1)
            ).astype(np.float16)
        in_maps.append(im)

    nc = _get_compiled(chunks)
    res = None
    for attempt in range(3):
        try:
            res = bass_utils.run_bass_kernel_spmd(
                nc, in_maps, core_ids=list(range(N_CORES)), trace=_maybe_trace()
            )
            break
        except Exception:
            # transient runtime/tunnel hiccups: retry a couple of times
            if attempt == 2:
                raise
            import time

            time.sleep(2.0)
    if res.exec_time_ns is not None:
        print(f"HW exec time: {res.exec_time_ns} ns")
        if res.instructions_and_trace is not None:
            print(f"trace: {res.instructions_and_trace[1]}")

    out = np.zeros((T, Dm), dtype=np.float32)
    for e in range(E):
        ids = order[starts[e] : starts[e + 1]]
        yT = res.results[e]["yT"]
        out[ids] = yT[:, : len(ids)].T.astype(np.float32)
    return out.reshape(B, S, Dm)

